# revision 58
# baseline (speedup 1.0000x reference)
"""GateRetention Trainium2 kernel (Bass/Tile), 8-core tensor-parallel, v2.

Sharding: core grid (batch b = core//4, head-group g = core%4); each core owns
4 heads (512 cols of the q/k/v/g projections, 512 rows of Wo) of one batch.
RMS-norm statistics AND the gate-logit projection (K-sharded over the 4 TP
cores) ride one AllReduce per token-half; out-proj partials summed on host.

v2 changes vs v1: fp16 projections in a single pass over x (x loaded once);
gt K-sharded (64 instead of 256 tensor instructions); AllReduce split into two
token-halves, each hidden under later compute; P3 folds the decay factor into
the A^T / k_nat evacuations (no separate vv tiles), computes the o-norm via
Square+accum_out straight from PSUM, folds rowfac+norm into one per-column
evac factor, and software-pipelines the out-proj one chunk behind retention.

kernel(**inputs) takes the FULL inputs from reference.setup_inputs() and
returns the FULL [B, T, DIM] fp32 output.
"""
import os
import sys

sys.path.insert(0, "/opt/trn_rl_repo")

import numpy as np

import concourse.bass as bass
import concourse.bacc as bacc
import concourse.tile as tile
import concourse.mybir as mybir
from concourse import bass_utils

F32 = mybir.dt.float32
F32R = mybir.dt.float32r
F16 = mybir.dt.float16
AX = mybir.AxisListType
ALU = mybir.AluOpType
ACTF = mybir.ActivationFunctionType

B, T, DIM = 2, 4096, 2048
H, HD = 16, 128
CS = 256
NCH = T // CS              # 16 chunks
EPS = 1e-5
GLN = 16.0
SCALE = HD ** -0.5
NCORE = 8
HPC = 4                    # heads per core
PCOLS = HPC * HD           # 512 cols per core
NBLK = T // 128            # 32 token blocks of 128
TSEG = T // 2              # 2048 tokens per AllReduce segment
NTS = TSEG // 512          # 4 token n-tiles per segment
SROWS = 3 + H              # sumsq q/k/v + 16 gt logit rows
VSH = 2.0 ** -4            # fp16 range shift on decayed tensors

DEBUG = bool(int(os.environ.get("GR_DEBUG", "0")))
TRACE = bool(int(os.environ.get("GR_TRACE", "0")))

_cache = {}


def _consts_np():
    """fp32 consts [128, 512]: identity | Lm | Om | Lc."""
    ident = np.eye(128, dtype=np.float32)
    jj, ii = np.meshgrid(np.arange(128), np.arange(128), indexing="ij")
    Lm = np.where(jj <= ii, -1.0 / GLN, 0.0).astype(np.float32)
    Om = np.full((128, 128), -1.0 / GLN, np.float32)
    # Lc: b' for even blocks = +1/GLN * sum_{j>i} sp_j (decay i -> block end)
    Lc = np.where(jj > ii, 1.0 / GLN, 0.0).astype(np.float32)
    return np.concatenate([ident, Lm, Om, Lc], axis=1)


def _consts16_np():
    """fp16 consts [128, 264]: identity | Um (tril ones) | ones."""
    ident = np.eye(128, dtype=np.float16)
    jj, ii = np.meshgrid(np.arange(128), np.arange(128), indexing="ij")
    Um = np.where(jj <= ii, 1.0, 0.0).astype(np.float16)
    ones = np.ones((128, 8), np.float16)
    return np.concatenate([ident, Um, ones], axis=1)


def build(debug=False):
    nc = bacc.Bacc("TRN2", target_bir_lowering=False, debug=False,
                   enable_asserts=False, num_devices=NCORE)

    # ---------------- I/O ----------------
    xT = nc.dram_tensor("xT", [DIM, T], F16, kind="ExternalInput").ap()
    xgt = nc.dram_tensor("xgt", [PCOLS, T], F16, kind="ExternalInput").ap()
    cgt = nc.dram_tensor("cgt", [PCOLS, T], F16, kind="ExternalInput").ap()
    wq = nc.dram_tensor("wq", [DIM, PCOLS], F16, kind="ExternalInput").ap()
    wk = nc.dram_tensor("wk", [DIM, PCOLS], F16, kind="ExternalInput").ap()
    wv = nc.dram_tensor("wv", [DIM, PCOLS], F16, kind="ExternalInput").ap()
    wg = nc.dram_tensor("wg", [DIM, PCOLS], F16, kind="ExternalInput").ap()
    wgt = nc.dram_tensor("wgt", [PCOLS, H], F16, kind="ExternalInput").ap()
    wo = nc.dram_tensor("wo", [PCOLS, DIM], F16, kind="ExternalInput").ap()
    consts = nc.dram_tensor("consts", [128, 512], F32R,
                            kind="ExternalInput").ap()
    c16 = nc.dram_tensor("c16", [128, 264], F16, kind="ExternalInput").ap()
    selT = nc.dram_tensor("selT", [SROWS, 8], F32, kind="ExternalInput").ap()
    out = nc.dram_tensor("out", [T, DIM], F16, kind="ExternalOutput").ap()

    def dbg(name, shape, dtype=F32):
        return nc.dram_tensor(name, shape, dtype, kind="ExternalOutput").ap()

    with tile.TileContext(nc) as tc:
        with (
            tc.tile_pool(name="const", bufs=1) as cpool,
            tc.tile_pool(name="wts", bufs=1) as wpool,
            tc.tile_pool(name="xstream", bufs=2) as xpool,
            tc.tile_pool(name="gstream", bufs=2) as gxpool,
            tc.tile_pool(name="evac", bufs=2) as epool,
            tc.tile_pool(name="persist", bufs=1) as ppool,
            tc.tile_pool(name="small", bufs=2) as spool,
            tc.tile_pool(name="ret", bufs=2) as rpool,
            tc.tile_pool(name="ps", bufs=1, space="PSUM") as psp,
        ):
            # PSUM buffers are whole 2KB banks: at most 8 concurrent.
            # big 2 + wide 2 + ops 2 + ops2 1 + tp 1 = 8.
            def ps_big():
                return psp.tile([128, 512], F32, tag="big", bufs=2,
                                name="psbig")

            def ps_wide(shape):
                return psp.tile(shape, F32, tag="wide", bufs=2, name="pswide")

            def ps_o(shape=None):
                return psp.tile(shape or [128, HD], F32, tag="ops", bufs=2,
                                name="pso")

            def ps_tp():
                return psp.tile([128, 128], F16, tag="tp", bufs=2, name="pstp")

            # ---------------- first x tile ahead of everything ----------
            def p1_inputs(nt):
                tok = slice(nt * 512, (nt + 1) * 512)
                xt = xpool.tile([128, 2, 8, 512], F16, tag="xt")
                nc.sync.dma_start(
                    xt[:], xT[:, tok].rearrange("(h k p) m -> p h k m",
                                                p=128, k=8))
                xg = gxpool.tile([128, 4, 512], F16, tag="xg")
                nc.sync.dma_start(
                    xg[:], xgt[:, tok].rearrange("(k p) m -> p k m", p=128))
                cg = gxpool.tile([128, 4, 512], F16, tag="cg")
                nc.sync.dma_start(
                    cg[:], cgt[:, tok].rearrange("(k p) m -> p k m", p=128))
                return xt, xg, cg

            # ---------------- constants ----------------
            cst = cpool.tile([128, 512], F32R, tag="consts")
            nc.sync.dma_start(cst[:], consts)
            ident32 = cst[:, 0:128].bitcast(F32)
            Lm = cst[:, 128:256]
            Om = cst[:, 256:384]
            Lc = cst[:, 384:512]
            cst16 = cpool.tile([128, 264], F16, tag="c16")
            nc.sync.dma_start(cst16[:], c16)
            i16 = cst16[:, 0:128]
            Um16 = cst16[:, 128:256]
            ones16 = cst16[:, 256:257]
            selt = cpool.tile([SROWS, 8], F32, tag="selt")
            nc.sync.dma_start(selt[:], selT)

            # ---------------- DRAM scratch (fp16) ----------------
            if debug:
                qT_s = dbg("dbg_qT", [PCOLS, T], F16)
                kT_s = dbg("dbg_kT", [PCOLS, T], F16)
                vN_s = dbg("dbg_vN", [T, PCOLS], F16)
                gT_s = dbg("dbg_gT", [PCOLS, T], F16)
            else:
                qT_s = nc.dram_tensor("qT_s", [PCOLS, T], F16,
                                      kind="Internal").ap()
                kT_s = nc.dram_tensor("kT_s", [PCOLS, T], F16,
                                      kind="Internal").ap()
                vN_s = nc.dram_tensor("vN_s", [T, PCOLS], F16,
                                      kind="Internal").ap()
                gT_s = nc.dram_tensor("gT_s", [PCOLS, T], F16,
                                      kind="Internal").ap()
            ss_in = [nc.dram_tensor(f"ss_in{s}", [SROWS, TSEG], F32,
                                    kind="Internal").ap() for s in range(2)]
            ss_out = [nc.dram_tensor(f"ss_out{s}", [SROWS, TSEG], F32,
                                     kind="Internal").ap() for s in range(2)]

            # ---------------- weights (parallel DMA queues so the first
            # x tile is not stuck behind 8MB of weights) ----------------
            def load_w(wdram, tag, eng):
                wt = wpool.tile([128, 16, PCOLS], F16, tag=tag)
                eng.dma_start(
                    wt[:], wdram.rearrange("(kt p) m -> p kt m", p=128))
                return wt

            wq_sb = load_w(wq, "wq", nc.sync)
            wk_sb = load_w(wk, "wk", nc.scalar)
            wg_sb = load_w(wg, "wg", nc.gpsimd)
            wv_sb = load_w(wv, "wv", nc.scalar)
            wgt_sb = wpool.tile([128, 4, H], F16, tag="wgt")
            nc.scalar.dma_start(wgt_sb[:],
                                wgt.rearrange("(kt p) m -> p kt m", p=128))

            # persistent P2 state
            vss = ppool.tile([128, NBLK], F32, tag="vss")
            gtd = ppool.tile([128, NCH, 2, HPC], F32R, tag="gtd")
            rf = ppool.tile([128, NCH, 2, HPC], F32, tag="rf")
            vf = ppool.tile([128, NCH, 2, HPC], F32, tag="vf")
            eS = ppool.tile([128, NCH - 1, HPC], F32, tag="eS")
            eSb7 = ppool.tile([128, HPC], F32, tag="eSb7")

            # =========================================================
            # P1: one pass over x -> q,k,g (T-layout), v (natural), gt
            # =========================================================
            def p1_ntile(nt, seg, pre=None):
                tok = slice(nt * 512, (nt + 1) * 512)
                ltok = slice((nt - seg * NTS) * 512, (nt - seg * NTS + 1) * 512)
                xt, xg, cg = pre if pre is not None else p1_inputs(nt)

                # q, k: T-layout + sumsq over channels
                for pi, (w_sb, sdram, row) in enumerate(
                        ((wq_sb, qT_s, 0), (wk_sb, kT_s, 1))):
                    ssps = None
                    for m in range(4):
                        msl = slice(m * 128, (m + 1) * 128)
                        ps = ps_big()
                        for kk in range(16):
                            nc.tensor.matmul(ps[:], w_sb[:, kk, msl],
                                             xt[:, kk // 8, kk % 8, :],
                                             start=(kk == 0), stop=(kk == 15))
                        ev = epool.tile([128, 512], F16, tag="ev", bufs=4)
                        if m % 2 == 0:
                            nc.vector.tensor_copy(ev[:], ps[:])
                            nc.sync.dma_start(
                                sdram[m * 128:(m + 1) * 128, tok], ev[:])
                        else:
                            nc.scalar.copy(ev[:], ps[:])
                            nc.scalar.dma_start(
                                sdram[m * 128:(m + 1) * 128, tok], ev[:])
                        sqt = epool.tile([128, 512], F16, tag="sq", bufs=3)
                        nc.scalar.activation(sqt[:], ps[:], ACTF.Square)
                        if m == 0:
                            ssps = ps_wide([1, 512])
                        nc.tensor.matmul(ssps[:1, :], ones16, sqt[:],
                                         start=(m == 0), stop=(m == 3))
                    ssev = spool.tile([1, 512], F32, tag="ssev", bufs=2)
                    nc.vector.tensor_copy(ssev[:], ssps[:1, :])
                    nc.sync.dma_start(ss_in[seg][row:row + 1, ltok], ssev[:])

                # g: T-layout, silu fused into evac
                for m in range(4):
                    msl = slice(m * 128, (m + 1) * 128)
                    ps = ps_big()
                    for kk in range(16):
                        nc.tensor.matmul(ps[:], wg_sb[:, kk, msl],
                                         xt[:, kk // 8, kk % 8, :],
                                         start=(kk == 0), stop=(kk == 15))
                    ev = epool.tile([128, 512], F16, tag="ev", bufs=4)
                    nc.scalar.activation(ev[:], ps[:], ACTF.Silu)
                    nc.scalar.dma_start(gT_s[m * 128:(m + 1) * 128, tok],
                                        ev[:])

                # v: natural layout + sumsq via accum
                for mt in range(4):
                    msl = slice(mt * 128, (mt + 1) * 128)
                    ps = ps_big()
                    for kk in range(16):
                        nc.tensor.matmul(ps[:], xt[:, kk // 8, kk % 8, msl],
                                         wv_sb[:, kk, :],
                                         start=(kk == 0), stop=(kk == 15))
                    ev = epool.tile([128, 512], F16, tag="ev", bufs=4)
                    sqv = epool.tile([128, 512], F16, tag="sq", bufs=3)
                    nc.scalar.activation(
                        sqv[:], ps[:], ACTF.Square,
                        accum_out=vss[:, nt * 4 + mt:nt * 4 + mt + 1])
                    if mt % 2 == 0:
                        nc.vector.tensor_copy(ev[:], ps[:])
                        nc.sync.dma_start(
                            vN_s[nt * 512 + mt * 128:nt * 512 + (mt + 1) * 128,
                                 :], ev[:])
                    else:
                        nc.scalar.copy(ev[:], ps[:])
                        nc.scalar.dma_start(
                            vN_s[nt * 512 + mt * 128:nt * 512 + (mt + 1) * 128,
                                 :], ev[:])

                # gt logits, K-sharded: this core's 512 rows of x+c
                gtps = ps_wide([H, 512])
                for kk in range(4):
                    nc.tensor.matmul(gtps[:H, :], wgt_sb[:, kk, :],
                                     xg[:, kk, :], start=(kk == 0), stop=False)
                for kk in range(4):
                    nc.tensor.matmul(gtps[:H, :], wgt_sb[:, kk, :],
                                     cg[:, kk, :], start=False,
                                     stop=(kk == 3))
                gev = spool.tile([H, 512], F32, tag="gev", bufs=2)
                nc.vector.tensor_copy(gev[:], gtps[:H, :])
                nc.sync.dma_start(ss_in[seg][3:3 + H, ltok], gev[:])

            def p1_seg_wrap(seg):
                # v sumsq: transpose this segment's 16 block-columns to a row
                vssT = ps_o([128, 128])
                nc.tensor.matmul(
                    vssT[:16, :],
                    vss[:, seg * 16:(seg + 1) * 16], ident32,
                    is_transpose=True)
                vssev = spool.tile([16, 128], F32, tag="vssev", bufs=2)
                nc.vector.tensor_copy(vssev[:], vssT[:16, :])
                nc.sync.dma_start(
                    ss_in[seg][2:3, :].rearrange("a (b c) -> (a b) c", c=128),
                    vssev[:])

            # =========================================================
            # P2 (per segment): scales + gate decays
            # =========================================================
            def p2_seg(seg):
                chs = slice(seg * 8, seg * 8 + 8)
                # gpsimd queue: only the AllReduce triggers live there, so
                # this wait-on-collective load can't block chunk prefetch
                srt = spool.tile([SROWS, TSEG], F32, tag="srt", bufs=1)
                nc.gpsimd.dma_start(srt[:], ss_out[seg])
                ssel = ppool.tile([128, 16, 8], F32, tag=f"ssel{seg}")
                for j in range(16):
                    tp = ps_o([128, 8])
                    nc.tensor.matmul(tp[:], srt[:, j * 128:(j + 1) * 128],
                                     selt[:], start=True, stop=True)
                    nc.vector.tensor_copy(ssel[:, j, :], tp[:])
                # rsn = (ms/DIM + EPS)^-0.5  (1/DIM folded into selT)
                rsn = spool.tile([128, 16, 3], F32, tag="rsn", bufs=2)
                nc.vector.tensor_scalar(rsn[:], ssel[:, :, 0:3], 1.0, EPS,
                                        ALU.mult, ALU.add)
                nc.scalar.activation(rsn[:], rsn[:], ACTF.Ln)
                nc.scalar.activation(rsn[:], rsn[:], ACTF.Exp, scale=-0.5)
                # rsq = rsn_q * SCALE / VSH ; skv = rsn_k * rsn_v * VSH
                # ([128, 8, 2] so even/odd blocks are plain slices)
                rsq = spool.tile([128, 8, 2], F32, tag="rsq", bufs=2)
                nc.vector.tensor_scalar(
                    rsq[:].rearrange("p a b -> p (a b)"), rsn[:, :, 0],
                    SCALE / VSH, None, ALU.mult)
                skv = spool.tile([128, 8, 2], F32, tag="skv", bufs=2)
                skv_f = skv[:].rearrange("p a b -> p (a b)")
                nc.vector.tensor_mul(skv_f, rsn[:, :, 1], rsn[:, :, 2])
                nc.vector.tensor_scalar(skv_f, skv_f, VSH, None, ALU.mult)
                # gtd = softplus(-z) = ln(1 + exp(-z)); selT folds the -1
                gt_view = gtd[:, chs].rearrange("p a b c -> p (a b) c")
                nc.scalar.activation(gt_view, ssel[:, :, 3:7], ACTF.Exp)
                nc.scalar.activation(gt_view, gt_view, ACTF.Ln, bias=1.0)

                # recentred decays, batched over the segment's 8 chunks.
                # rf = exp(b')*rsq ; vf = exp(-b')*skv per block; each small
                # PSUM is consumed before the next is filled (2 bufs).
                for b01, tri in ((0, Lc), (1, Lm)):
                    pp = ps_o([128, 8, HPC])
                    nc.tensor.matmul(pp[:], tri, gtd[:, chs, b01, :],
                                     start=True, stop=True)
                    ex = spool.tile([128, 8, HPC], F32, tag="p2e", bufs=4)
                    nc.scalar.activation(ex[:], pp[:], ACTF.Exp)
                    nc.vector.tensor_tensor(
                        rf[:, chs, b01, :], ex[:],
                        rsq[:, :, b01].unsqueeze(2).to_broadcast(
                            [128, 8, HPC]), ALU.mult)
                    ex2 = spool.tile([128, 8, HPC], F32, tag="p2e", bufs=4)
                    nc.scalar.activation(ex2[:], pp[:], ACTF.Exp, scale=-1.0)
                    nc.vector.tensor_tensor(
                        vf[:, chs, b01, :], ex2[:],
                        skv[:, :, b01].unsqueeze(2).to_broadcast(
                            [128, 8, HPC]), ALU.mult)
                ptv = ps_o([128, 8, HPC])
                nc.tensor.matmul(ptv[:], Om, gtd[:, chs, 1, :],
                                 start=True, stop=False)
                nc.tensor.matmul(ptv[:, 0:7, :], Om,
                                 gtd[:, seg * 8 + 1:seg * 8 + 8, 0, :],
                                 start=False, stop=True, skip_group_check=True)
                if seg == 0:
                    # cols 0..6 complete; col 7 lacks block 16 (next segment)
                    nc.scalar.activation(eS[:, 0:8, :], ptv[:], ACTF.Exp)
                else:
                    nc.scalar.activation(eS[:, 8:15, :], ptv[:, 0:7, :],
                                         ACTF.Exp)
                    pb = ps_o([128, HPC])
                    nc.tensor.matmul(pb[:], Om, gtd[:, 8, 0, :],
                                     start=True, stop=True)
                    nc.scalar.activation(eSb7[:], pb[:], ACTF.Exp)

            # =========================================================
            # P3: retention + gating + out-proj, out-proj 1 chunk behind
            # =========================================================
            # reuse wq's SBUF (P1 done by the time the load lands)
            wo_sb = wpool.tile([128, HPC, DIM], F16, tag="wq")

            S_prev = [None] * HPC

            def p3_retention(ch, inject):
                tok = slice(ch * CS, (ch + 1) * CS)
                qc = rpool.tile([128, HPC, CS], F16, tag="qc", bufs=3)
                kc = rpool.tile([128, HPC, CS], F16, tag="kc", bufs=3)
                for t_, s_ in ((qc, qT_s), (kc, kT_s)):
                    nc.sync.dma_start(
                        t_[:], s_[:, tok].rearrange("(h p) m -> p h m", p=128))
                vcn, sg = [], []
                for b01 in range(2):
                    bt = slice(ch * CS + b01 * 128, ch * CS + b01 * 128 + 128)
                    vt = rpool.tile([128, PCOLS], F16, tag="vcn", bufs=4)
                    nc.sync.dma_start(vt[:], vN_s[bt, :])
                    vcn.append(vt)
                    gt_ = rpool.tile([128, HPC, 128], F16, tag="gch", bufs=4)
                    nc.sync.dma_start(
                        gt_[:], gT_s[:, bt].rearrange("(h p) m -> p h m",
                                                      p=128))
                    sg.append(gt_)
                if ch == 8:
                    # deferred cross-segment state decay (block 16 part)
                    for hl in range(HPC):
                        Sn = rpool.tile([128, HD], F16, tag=f"S{hl}", bufs=3)
                        nc.vector.tensor_scalar(
                            Sn[:], S_prev[hl][:], eSb7[:, hl:hl + 1], None,
                            ALU.mult)
                        S_prev[hl] = Sn
                S_old = list(S_prev)
                # phase A: k transposes (decayed), A^T (masked, decayed),
                # state update
                at0s, at1s = [], []
                for hl in range(HPC):
                    knat = []
                    if ch < NCH - 1:
                        for b01 in range(2):
                            bsl = slice(b01 * 128, b01 * 128 + 128)
                            tpk = ps_tp()
                            nc.tensor.transpose(tpk[:], kc[:, hl, bsl],
                                                i16[:])
                            kn = rpool.tile([128, 128], F16, tag="knat",
                                            bufs=4)
                            nc.scalar.mul(kn[:], tpk[:],
                                          vf[:, ch, b01, hl:hl + 1])
                            knat.append(kn)
                    atps = ps_wide([128, 384])
                    nc.tensor.matmul(atps[:, 0:256], kc[:, hl, 0:128],
                                     qc[:, hl, :], start=True, stop=True)
                    nc.tensor.matmul(atps[:, 256:384], kc[:, hl, 128:256],
                                     qc[:, hl, 128:256], start=True, stop=True,
                                     skip_group_check=True)
                    at0 = rpool.tile([128, CS], F16, tag="at0", bufs=5)
                    nc.vector.tensor_scalar(at0[:], atps[:, 0:256],
                                            vf[:, ch, 0, hl:hl + 1], None,
                                            ALU.mult)
                    nc.vector.tensor_mul(at0[:, 0:128], at0[:, 0:128], Um16)
                    at1 = rpool.tile([128, 128], F16, tag="at1", bufs=5)
                    nc.vector.tensor_scalar(at1[:], atps[:, 256:384],
                                            vf[:, ch, 1, hl:hl + 1], None,
                                            ALU.mult)
                    nc.vector.tensor_mul(at1[:], at1[:], Um16)
                    at0s.append(at0)
                    at1s.append(at1)
                    if ch < NCH - 1:
                        sps = ps_o()
                        nc.tensor.matmul(sps[:], knat[0][:],
                                         vcn[0][:, hl * 128:(hl + 1) * 128],
                                         start=True, stop=False)
                        nc.tensor.matmul(sps[:], knat[1][:],
                                         vcn[1][:, hl * 128:(hl + 1) * 128],
                                         start=False, stop=True)
                        S_cur = rpool.tile([128, HD], F16, tag=f"S{hl}",
                                           bufs=3)
                        if ch > 0:
                            stmp = rpool.tile([128, HD], F32, tag="stmp",
                                              bufs=2)
                            nc.vector.tensor_add(stmp[:], S_prev[hl][:],
                                                 sps[:])
                            nc.vector.tensor_scalar(
                                S_cur[:], stmp[:], eS[:, ch, hl:hl + 1], None,
                                ALU.mult)
                        else:
                            nc.vector.tensor_scalar(
                                S_cur[:], sps[:], eS[:, ch, hl:hl + 1], None,
                                ALU.mult)
                        S_prev[hl] = S_cur
                return qc, vcn, sg, at0s, at1s, S_old

            def p3_phaseBC(ch, st, inject):
                qc, vcn, sg, at0s, at1s, S_old = st
                # o per ci-half; raw o evacuated fp16, then the
                # rowfac+rmsnorm factor F applied as one per-column multiply
                o_r = rpool.tile([128, 2 * HPC, HD], F16, tag="o_r")
                o_n = rpool.tile([128, 2 * HPC, HD], F16, tag="o_n")
                msq = rpool.tile([128, 2, HPC], F32, tag="msq")
                for ci in range(2):
                    csl = slice(ci * 128, ci * 128 + 128)
                    for hl in range(HPC):
                        mms = [(at0s[hl][:, csl],
                                vcn[0][:, hl * 128:(hl + 1) * 128])]
                        if ci == 1:
                            mms.append((at1s[hl][:],
                                        vcn[1][:, hl * 128:(hl + 1) * 128]))
                        if ch > 0:
                            mms.append((qc[:, hl, csl], S_old[hl][:]))
                        ops = ps_o()
                        for i, (lh, rh) in enumerate(mms):
                            nc.tensor.matmul(ops[:], lh, rh, start=(i == 0),
                                             stop=(i == len(mms) - 1))
                        osl = o_r[:, ci * HPC + hl, :]
                        nc.scalar.copy(osl, ops[:])
                        sqs = rpool.tile([128, HD], F32, tag="sqs", bufs=2)
                        nc.vector.scalar_tensor_tensor(
                            sqs[:], osl, 1.0, osl, op0=ALU.mult, op1=ALU.mult,
                            accum_out=msq[:, ci, hl:hl + 1])
                # F = rf * (rf^2 * msq / HD + EPS)^-0.5, one batch per chunk,
                # then a single broadcast multiply for all 8 (ci, head) tiles
                Ft = rpool.tile([128, 2, HPC], F32, tag="Ft", bufs=2)
                rfc = rf[:, ch, :, :]
                nc.vector.tensor_mul(Ft[:], rfc, rfc)
                nc.vector.tensor_mul(Ft[:], Ft[:], msq[:])
                nc.vector.tensor_scalar(Ft[:], Ft[:], 1.0 / HD, EPS,
                                        ALU.mult, ALU.add)
                nc.vector.reciprocal(Ft[:], Ft[:])
                nc.scalar.activation(Ft[:], Ft[:], ACTF.Sqrt)
                nc.vector.tensor_mul(Ft[:], Ft[:], rfc)
                nc.vector.tensor_tensor(
                    o_n[:], o_r[:],
                    Ft[:].rearrange("p a b -> p (a b)").unsqueeze(
                        2).to_broadcast([128, 2 * HPC, HD]), ALU.mult)
                return o_n

            def p3_output_units(ch, o_n, sg):
                """Output work for chunk ch as closures, injected between the
                next chunk's PSUM groups so boundary LDWs hide under them."""
                go_st = rpool.tile([128, HPC, CS], F16, tag="go_st")
                units = []

                def mk_trp(hl):
                    def f():
                        for b01 in range(2):
                            trp = ps_tp()
                            nc.tensor.transpose(
                                trp[:], o_n[:][:, b01 * HPC + hl, :], i16[:])
                            bsl = slice(b01 * 128, b01 * 128 + 128)
                            nc.vector.tensor_mul(
                                go_st[:, hl, bsl], trp[:], sg[b01][:, hl, :])
                    return f

                def mk_op(m01, n):
                    def f():
                        msl = slice(m01 * 128, m01 * 128 + 128)
                        ps = ps_big()
                        nsl = slice(n * 512, (n + 1) * 512)
                        for k in range(HPC):
                            nc.tensor.matmul(ps[:], go_st[:, k, msl],
                                             wo_sb[:, k, nsl],
                                             start=(k == 0),
                                             stop=(k == HPC - 1))
                        oo = epool.tile([128, 512], F16, tag="oo", bufs=4)
                        if n % 2 == 0:
                            nc.vector.tensor_copy(oo[:], ps[:])
                            nc.sync.dma_start(
                                out[ch * CS + m01 * 128:
                                    ch * CS + m01 * 128 + 128, nsl], oo[:])
                        else:
                            nc.scalar.copy(oo[:], ps[:])
                            nc.scalar.dma_start(
                                out[ch * CS + m01 * 128:
                                    ch * CS + m01 * 128 + 128, nsl], oo[:])
                    return f

                for hl in range(HPC):
                    units.append(mk_trp(hl))
                for m01 in range(2):
                    for n in range(DIM // 512):
                        units.append(mk_op(m01, n))
                return units

            # ------------------ emission order ------------------
            for nt in range(NTS):
                p1_ntile(nt, 0)
            p1_seg_wrap(0)
            nc.gpsimd.collective_compute(
                "AllReduce", ALU.add,
                replica_groups=[[0, 1, 2, 3], [4, 5, 6, 7]],
                ins=[ss_in[0].opt()], outs=[ss_out[0].opt()],
            )
            for nt in range(NTS, 2 * NTS):
                p1_ntile(nt, 1)
                if nt == NTS + 1:
                    # P2a scalar/vector chain hides under remaining P1b
                    p2_seg(0)
            p1_seg_wrap(1)
            nc.gpsimd.collective_compute(
                "AllReduce", ALU.add,
                replica_groups=[[0, 1, 2, 3], [4, 5, 6, 7]],
                ins=[ss_in[1].opt()], outs=[ss_out[1].opt()],
            )
            nc.sync.dma_start(wo_sb[:],
                              wo.rearrange("(h p) m -> p h m", p=128))
            pend_units = []
            for ch in range(NCH):
                if ch == 5:
                    # P2b hides under mid-P3a retention
                    p2_seg(1)
                st = p3_retention(ch, None)
                for u in pend_units:
                    u()
                o_n = p3_phaseBC(ch, st, None)
                pend_units = p3_output_units(ch, o_n, st[2])
            for u in pend_units:
                u()

    nc.compile()
    return nc


def _prep_inputs(x, c, Wq, Wk, Wv, Wg, Wgt, Wo):
    """Build the 8 per-core input maps (host-side sharding / layout)."""
    consts = np.ascontiguousarray(_consts_np())
    c16 = np.ascontiguousarray(_consts16_np())
    in_maps = []
    xTs = [np.ascontiguousarray(x[b].T.astype(np.float16)) for b in range(B)]
    cTs = [np.ascontiguousarray(c[b].T.astype(np.float16)) for b in range(B)]
    Wgt16 = Wgt.astype(np.float16)
    for core in range(NCORE):
        b, g = core // 4, core % 4
        cols = slice(g * PCOLS, (g + 1) * PCOLS)
        sel = np.zeros((SROWS, 8), np.float32)
        for j in range(3):
            sel[j, j] = 1.0 / DIM
        for jj in range(HPC):
            sel[3 + 4 * g + jj, 3 + jj] = -1.0
        in_maps.append({
            "xT": xTs[b],
            "xgt": np.ascontiguousarray(xTs[b][cols.start:cols.stop, :]),
            "cgt": np.ascontiguousarray(cTs[b][cols.start:cols.stop, :]),
            "wq": np.ascontiguousarray(Wq[:, cols]).astype(np.float16),
            "wk": np.ascontiguousarray(Wk[:, cols]).astype(np.float16),
            "wv": np.ascontiguousarray(Wv[:, cols]).astype(np.float16),
            "wg": np.ascontiguousarray(Wg[:, cols]).astype(np.float16),
            "wgt": np.ascontiguousarray(Wgt16[cols, :]),
            "wo": np.ascontiguousarray(Wo[cols, :]).astype(np.float16),
            "consts": consts,
            "c16": c16,
            "selT": sel,
        })
    return in_maps


def kernel(x, c, Wq, Wk, Wv, Wg, Wgt, Wo, _want_results=False):
    key = "nc_dbg" if DEBUG else "nc"
    if key not in _cache:
        _cache[key] = build(debug=DEBUG)
    nc = _cache[key]
    in_maps = _prep_inputs(np.asarray(x, np.float32), np.asarray(c, np.float32),
                           np.asarray(Wq, np.float32), np.asarray(Wk, np.float32),
                           np.asarray(Wv, np.float32), np.asarray(Wg, np.float32),
                           np.asarray(Wgt, np.float32), np.asarray(Wo, np.float32))
    res = bass_utils.run_bass_kernel_spmd(
        nc, in_maps, core_ids=list(range(NCORE)), trace=TRACE)
    out = np.zeros((B, T, DIM), np.float32)
    for core in range(NCORE):
        out[core // 4] += res.results[core]["out"].astype(np.float32)
    if _want_results:
        return out, res
    return out
